# revision 18
# baseline (speedup 1.0000x reference)
"""Trainium2 kernel for EntropyRegularizedVQ (vq_codebook).

Contract: kernel(z_real, z_imag, weight) -> (z_q_c, loss_sample, indices, batch_entropy)
matching reference.py numerics. Self-contained: hardcoded shapes/sharding.

Strategy
--------
Device (8 NeuronCores, data-parallel over tokens, codebook replicated):
  scores[t, j] = z_t . w_j computed as fp16 matmul (fp32 PSUM accumulate).
  The codebook is pre-sorted by ||w||^2 and laid out so that a contiguous
  binary max-fold of the 8192 score slots yields, per token, the max score
  of each GROUP of 16 codes with adjacent ||w||^2.  ACT evacuates PSUM
  (fp32 -> fp16), DVE does the 4-level fold, DMA returns G [tokens, 512].

Host:
  A rigorous interval filter (fp16 ulp + matmul-error margin + per-group
  ||w||^2 range) selects ~1.1 candidate groups per token; those 16-code
  groups are rescored exactly in fp64 to get the argmin index.  All other
  outputs (z_q, loss, histogram entropy) are cheap host math.
"""

import sys

for _p in ("/opt/trn_rl_repo", "/root/.axon_site/_ro/trn_rl_repo"):
    if _p not in sys.path:
        sys.path.insert(0, _p)

import numpy as np

N_TOKENS = 32768
DIM = 64
D2 = 2 * DIM  # 128
K = 8192
N_CORES = 8
TOK_PER_CORE = N_TOKENS // N_CORES  # 4096
TILES_PER_CORE = TOK_PER_CORE // 128  # 32
GROUP = 4  # codes per group (adjacent in ||w||^2)
NGROUPS = K // GROUP  # 2048
CHUNK = 512  # matmul moving free dim (one PSUM bank of fp32)
NCHUNKS = K // CHUNK  # 16
QUAD = 4  # chunks per PSUM tile (4 banks)
ACT_CHUNKS = 12  # chunks evacuated by ACT; the rest are DVE chain-folded

# margin (score scale): 8-sigma fp16 matmul error + fp16 ulp at |s|<=16
MARGIN = 0.015

_cached = {}


def _build_nc(act_chunks=ACT_CHUNKS):
    """Hand-scheduled raw-bass SPMD program (one core's view).

    Engine streams (per 128-token tile):
      PE:   4 quads x 4 matmuls (N=512) -> 2 ping-pong PSUM tiles [128,2048]
      ACT:  quads 0-2: copy PSUM fp32 -> xa fp16 (evacuation)
      DVE:  quad 3: chain-folds max(psum, xa[c-2]) into xa slots 12-15,
            then 3 binary folds xa[8192] -> G[1024], signal DMA
      SYNC: g_out DMA per tile
    """
    from contextlib import ExitStack

    import concourse.bacc as bacc
    import concourse.mybir as mybir

    T = TILES_PER_CORE
    f16 = mybir.dt.float16
    f32 = mybir.dt.float32

    nc = bacc.Bacc()
    zT_in = nc.declare_dram_parameter("zT", [D2, TOK_PER_CORE], f16, isOutput=False)
    wT_in = nc.declare_dram_parameter("wT", [D2, K], f16, isOutput=False)
    g_out = nc.declare_dram_parameter("g_out", [TOK_PER_CORE, NGROUPS], f16, isOutput=True)

    ctx = ExitStack()
    zt_all = ctx.enter_context(nc.sbuf_tensor("zt_all", [D2, TOK_PER_CORE], f16))
    wt_all = ctx.enter_context(nc.sbuf_tensor("wt_all", [D2, K], f16))
    xa = [
        ctx.enter_context(nc.sbuf_tensor(f"xa{i}", [128, K], f16)) for i in range(3)
    ]
    f1 = ctx.enter_context(nc.sbuf_tensor("f1", [128, K // 2], f16))
    gt = [
        ctx.enter_context(nc.sbuf_tensor(f"gt{i}", [128, NGROUPS], f16))
        for i in range(3)
    ]
    NPS = 4  # PSUM rotation depth: 4 duo-buffers x 2 banks = all 8 banks
    DUO = 2 * CHUNK  # 1024 fp32 = 2 banks
    ps = [
        ctx.enter_context(nc.psum_tensor(f"ps{i}", [128, DUO], f32))
        for i in range(NPS)
    ]

    s_load = ctx.enter_context(nc.semaphore("s_load"))
    s_mm = ctx.enter_context(nc.semaphore("s_mm"))
    s_act = ctx.enter_context(nc.semaphore("s_act"))
    s_chain = ctx.enter_context(nc.semaphore("s_chain"))
    s_f1 = ctx.enter_context(nc.semaphore("s_f1"))
    s_g = ctx.enter_context(nc.semaphore("s_g"))
    s_dma = ctx.enter_context(nc.semaphore("s_dma"))

    # duo d of each tile: 0-4 evacuated by ACT into xa; 5-7 (chunks 10-15)
    # chain-folded by DVE straight into f1 slots (the f1 pairing is
    # (chunk c, chunk c-8), distance 4096 positions = 2*NGROUPS: group-safe)
    ACT_DUOS = 5

    with nc.Block() as block:

        @block.tensor
        def _(eng):
            eng.wait_ge(s_load, 32)
            for t in range(T):
                ztile = zt_all[:, t * 128 : (t + 1) * 128]
                for d in range(8):
                    D = 8 * t + d
                    if D >= NPS:
                        # wait for the consumer of duo D-NPS (same buffer)
                        Dp = D - NPS
                        tp, dp = divmod(Dp, 8)
                        if dp < ACT_DUOS:
                            eng.wait_ge(s_act, ACT_DUOS * tp + dp + 1)
                        else:
                            eng.wait_ge(s_chain, 3 * tp + (dp - ACT_DUOS) + 1)
                    for c2 in range(2):
                        c = d * 2 + c2
                        mm = nc.tensor.matmul(
                            ps[D % NPS][:, c2 * CHUNK : (c2 + 1) * CHUNK],
                            ztile,
                            wt_all[:, c * CHUNK : (c + 1) * CHUNK],
                            start=True,
                            stop=True,
                        )
                    mm.then_inc(s_mm, 1)

        @block.scalar
        def _(eng):
            for t in range(T):
                x = xa[t % 3]
                if t >= 3:
                    eng.wait_ge(s_f1, t - 2)
                for d in range(ACT_DUOS):
                    D = 8 * t + d
                    eng.wait_ge(s_mm, D + 1)
                    nc.scalar.copy(
                        x[:, d * DUO : (d + 1) * DUO], ps[D % NPS][:]
                    ).then_inc(s_act, 1)

        @block.vector
        def _(eng):
            for t in range(T):
                x = xa[t % 3]
                # chain duos 5-7: psum chunks (10+2k,11+2k) onto xa chunks
                # (2+2k,3+2k), writing f1 slots directly
                for k in range(3):
                    D = 8 * t + 5 + k
                    eng.wait_ge(s_mm, D + 1)
                    eng.wait_ge(s_act, ACT_DUOS * t + 2 + k)
                    nc.vector.tensor_max(
                        f1[:, (1 + k) * DUO : (2 + k) * DUO],
                        ps[D % NPS][:],
                        x[:, (1 + k) * DUO : (2 + k) * DUO],
                    ).then_inc(s_chain, 1)
                eng.wait_ge(s_act, ACT_DUOS * (t + 1))  # xa slots 0,1,8,9 ready
                # remaining f1 pairs: (chunks 0,1) vs (chunks 8,9)
                nc.vector.tensor_max(
                    f1[:, :DUO], x[:, :DUO], x[:, 8 * CHUNK : 8 * CHUNK + DUO]
                ).then_inc(s_f1, 1)
                if t >= 3:
                    eng.wait_ge(s_dma, 16 * (t - 2))
                nc.vector.tensor_max(
                    gt[t % 3][:], f1[:, :NGROUPS], f1[:, NGROUPS:]
                ).then_inc(s_g, 1)

        @block.sync
        def _(eng):
            eng.dma_start(zt_all[:], zT_in[:]).then_inc(s_load, 16)
            eng.dma_start(wt_all[:], wT_in[:]).then_inc(s_load, 16)
            for t in range(T):
                eng.wait_ge(s_g, t + 1)
                eng.dma_start(
                    g_out[t * 128 : (t + 1) * 128, :], gt[t % 3][:]
                ).then_inc(s_dma, 16)

    nc.finalize()
    ctx.close()
    return nc


def _build_nc_tile(act_chunks=ACT_CHUNKS):
    """Tile-scheduled variant (kept for A/B comparison)."""
    import concourse.bacc as bacc
    import concourse.mybir as mybir
    from concourse.tile import TileContext

    nc = bacc.Bacc()
    zT_in = nc.declare_dram_parameter(
        "zT", [D2, TOK_PER_CORE], mybir.dt.float16, isOutput=False
    )
    wT_in = nc.declare_dram_parameter("wT", [D2, K], mybir.dt.float16, isOutput=False)
    g_out = nc.declare_dram_parameter(
        "g_out", [TOK_PER_CORE, NGROUPS], mybir.dt.float16, isOutput=True
    )

    with TileContext(nc) as tc:
        with (
            tc.tile_pool(name="static", bufs=1) as stat,
            tc.tile_pool(name="xa", bufs=3) as xpool,
            tc.tile_pool(name="f1", bufs=2) as fpool1,
            tc.tile_pool(name="f2", bufs=2) as fpool2,
            tc.tile_pool(name="g", bufs=2) as gpool,
            tc.tile_pool(name="ps", bufs=2, space="PSUM") as pspool,
        ):
            zt_all = stat.tile([D2, TOK_PER_CORE], mybir.dt.float16)
            wt_all = stat.tile([D2, K], mybir.dt.float16)
            nc.sync.dma_start(zt_all[:], zT_in[:])
            nc.sync.dma_start(wt_all[:], wT_in[:])

            for t in range(TILES_PER_CORE):
                ztile = zt_all[:, t * 128 : (t + 1) * 128]
                xa = xpool.tile([128, K], mybir.dt.float16)
                for quad in range(NCHUNKS // QUAD):
                    pq = pspool.tile([128, QUAD * CHUNK], mybir.dt.float32)
                    for c4 in range(QUAD):
                        c = quad * QUAD + c4
                        nc.tensor.matmul(
                            pq[:, c4 * CHUNK : (c4 + 1) * CHUNK],
                            ztile,
                            wt_all[:, c * CHUNK : (c + 1) * CHUNK],
                            start=True,
                            stop=True,
                        )
                    q0 = quad * QUAD  # first chunk id of the quad
                    if q0 + QUAD <= act_chunks:
                        nc.scalar.copy(
                            xa[:, q0 * CHUNK : (q0 + QUAD) * CHUNK], pq[:]
                        )
                    else:
                        # ACT takes chunks < act_chunks; DVE chain-folds the rest:
                        # xa[c] = max(psum_c, xa[c-1])
                        if q0 < act_chunks:
                            nc.scalar.copy(
                                xa[:, q0 * CHUNK : act_chunks * CHUNK],
                                pq[:, : (act_chunks - q0) * CHUNK],
                            )
                        # merge distance must be a multiple of NGROUPS positions
                        # (2 chunks) to keep group classes aligned
                        step = NGROUPS // CHUNK  # chunks per group period
                        for c in range(max(q0, act_chunks), q0 + QUAD):
                            nc.vector.tensor_max(
                                xa[:, c * CHUNK : (c + 1) * CHUNK],
                                pq[:, (c - q0) * CHUNK : (c - q0 + 1) * CHUNK],
                                xa[:, (c - step) * CHUNK : (c - step + 1) * CHUNK],
                            )
                f1 = fpool1.tile([128, K // 2], mybir.dt.float16)
                nc.vector.tensor_max(f1[:], xa[:, : K // 2], xa[:, K // 2 :])
                f2 = fpool2.tile([128, K // 4], mybir.dt.float16)
                nc.vector.tensor_max(f2[:], f1[:, : K // 4], f1[:, K // 4 :])
                g = gpool.tile([128, NGROUPS], mybir.dt.float16)
                nc.vector.tensor_max(g[:], f2[:, :NGROUPS], f2[:, NGROUPS:])
                nc.sync.dma_start(g_out[t * 128 : (t + 1) * 128, :], g[:])

    nc.finalize()
    return nc


def _get_nc():
    if "nc" not in _cached:
        _cached["nc"] = _build_nc()
    return _cached["nc"]


def _host_prep(z_real, z_imag, weight):
    z = np.concatenate(
        [np.asarray(z_real, np.float32), np.asarray(z_imag, np.float32)], axis=1
    )  # [N, 128]
    w = np.asarray(weight, np.float32)
    y64 = (w.astype(np.float64) ** 2).sum(1)  # [K]
    order = np.argsort(y64, kind="stable")
    ws = w[order]  # [K, 128] sorted by ||w||^2
    ys = y64[order]

    u = np.arange(K)
    slots = (u % GROUP) * NGROUPS + (u // GROUP)  # code u -> score slot
    wT_dev = np.empty((D2, K), np.float16)
    wT_dev[:, slots] = ws.T.astype(np.float16)
    zT = np.ascontiguousarray(z.T).astype(np.float16)  # [128, N]
    return z, w, order, ws, ys, wT_dev, zT


def _device_groupmax(nc, zT, wT_dev):
    from concourse.bass_utils import run_bass_kernel_spmd

    in_maps = [
        {
            "zT": np.ascontiguousarray(
                zT[:, c * TOK_PER_CORE : (c + 1) * TOK_PER_CORE]
            ),
            "wT": wT_dev,
        }
        for c in range(N_CORES)
    ]
    res = run_bass_kernel_spmd(nc, in_maps, list(range(N_CORES)))
    G = np.concatenate([r["g_out"] for r in res.results], axis=0)
    return G.astype(np.float32)  # [N, 512]


def _host_select(z, order, ws, ys, G):
    """Filter candidate groups per token and rescore exactly in fp64."""
    n = z.shape[0]
    y_grp = ys.reshape(NGROUPS, GROUP)
    y_lo = y_grp[:, 0]  # sorted ascending within group
    y_hi = y_grp[:, -1]

    U = G + MARGIN - (y_lo * 0.5)[None, :]
    L = G - MARGIN - (y_hi * 0.5)[None, :]
    bestL = L.max(axis=1)
    cand = U >= bestL[:, None]

    tok_idx, grp_idx = np.nonzero(cand)
    o = np.argsort(grp_idx, kind="stable")
    tok_o = tok_idx[o]
    grp_o = grp_idx[o]
    bounds = np.searchsorted(grp_o, np.arange(NGROUPS + 1))

    zf = z.astype(np.float64)
    wsf = ws.astype(np.float64)
    npairs = tok_o.shape[0]
    pair_val = np.empty(npairs, np.float64)
    pair_j = np.empty(npairs, np.int64)

    for g in range(NGROUPS):
        s0, s1 = bounds[g], bounds[g + 1]
        if s0 == s1:
            continue
        toks = tok_o[s0:s1]
        lo, hi = GROUP * g, GROUP * (g + 1)
        sc = zf[toks] @ wsf[lo:hi].T - 0.5 * ys[lo:hi]
        mx = sc.max(axis=1)
        orig = order[lo:hi]
        tied = sc == mx[:, None]
        jj = np.where(tied, orig[None, :], np.iinfo(np.int64).max).min(axis=1)
        pair_val[s0:s1] = mx
        pair_j[s0:s1] = jj

    best_val = np.full(n, -np.inf)
    np.maximum.at(best_val, tok_o, pair_val)
    is_best = pair_val == best_val[tok_o]
    best_j = np.full(n, np.iinfo(np.int64).max)
    np.minimum.at(best_j, tok_o[is_best], pair_j[is_best])
    return best_j.astype(np.int64)


def kernel(z_real, z_imag, weight):
    z, w, order, ws, ys, wT_dev, zT = _host_prep(z_real, z_imag, weight)
    nc = _get_nc()
    G = _device_groupmax(nc, zT, wT_dev)
    idx = _host_select(z, order, ws, ys, G)

    z_q = w[idx]  # [N, 128] fp32
    z_q_c = (z_q[:, :DIM] + 1j * z_q[:, DIM:]).astype(np.complex64)
    diff = z_q.astype(np.float64) - z.astype(np.float64)
    loss_sample = (1.25 * (diff**2).mean(axis=1)).astype(np.float32)
    indices = idx.astype(np.int32)
    counts = np.bincount(idx, minlength=K).astype(np.float64)
    avg_probs = counts / z.shape[0]
    batch_entropy = np.float32(-(avg_probs * np.log(avg_probs + 1e-10)).sum())
    return z_q_c, loss_sample, indices, batch_entropy


# revision 19
# speedup vs baseline: 1.0941x; 1.0941x over previous
"""Trainium2 kernel for EntropyRegularizedVQ (vq_codebook).

Contract: kernel(z_real, z_imag, weight) -> (z_q_c, loss_sample, indices, batch_entropy)
matching reference.py numerics. Self-contained: hardcoded shapes/sharding.

Strategy
--------
Device (8 NeuronCores, data-parallel over tokens, codebook replicated):
  scores[t, j] = z_t . w_j computed as fp16 matmul (fp32 PSUM accumulate).
  The codebook is pre-sorted by ||w||^2 and laid out so that a contiguous
  binary max-fold of the 8192 score slots yields, per token, the max score
  of each GROUP of 16 codes with adjacent ||w||^2.  ACT evacuates PSUM
  (fp32 -> fp16), DVE does the 4-level fold, DMA returns G [tokens, 512].

Host:
  A rigorous interval filter (fp16 ulp + matmul-error margin + per-group
  ||w||^2 range) selects ~1.1 candidate groups per token; those 16-code
  groups are rescored exactly in fp64 to get the argmin index.  All other
  outputs (z_q, loss, histogram entropy) are cheap host math.
"""

import sys

for _p in ("/opt/trn_rl_repo", "/root/.axon_site/_ro/trn_rl_repo"):
    if _p not in sys.path:
        sys.path.insert(0, _p)

import numpy as np

N_TOKENS = 32768
DIM = 64
D2 = 2 * DIM  # 128
K = 8192
N_CORES = 8
TOK_PER_CORE = N_TOKENS // N_CORES  # 4096
TILES_PER_CORE = TOK_PER_CORE // 128  # 32
GROUP = 4  # codes per group (adjacent in ||w||^2)
NGROUPS = K // GROUP  # 2048
CHUNK = 512  # matmul moving free dim (one PSUM bank of fp32)
NCHUNKS = K // CHUNK  # 16
QUAD = 4  # chunks per PSUM tile (4 banks)
ACT_CHUNKS = 12  # chunks evacuated by ACT; the rest are DVE chain-folded

# margin (score scale): 8-sigma fp16 matmul error + fp16 ulp at |s|<=16
MARGIN = 0.015

_cached = {}


def _build_nc(act_chunks=ACT_CHUNKS):
    """Hand-scheduled raw-bass SPMD program (one core's view).

    Engine streams (per 128-token tile):
      PE:   4 quads x 4 matmuls (N=512) -> 2 ping-pong PSUM tiles [128,2048]
      ACT:  quads 0-2: copy PSUM fp32 -> xa fp16 (evacuation)
      DVE:  quad 3: chain-folds max(psum, xa[c-2]) into xa slots 12-15,
            then 3 binary folds xa[8192] -> G[1024], signal DMA
      SYNC: g_out DMA per tile
    """
    from contextlib import ExitStack

    import concourse.bacc as bacc
    import concourse.mybir as mybir

    T = TILES_PER_CORE
    f16 = mybir.dt.float16
    f32 = mybir.dt.float32

    nc = bacc.Bacc()
    zT_in = nc.declare_dram_parameter("zT", [D2, TOK_PER_CORE], f16, isOutput=False)
    wT_in = nc.declare_dram_parameter("wT", [D2, K], f16, isOutput=False)
    g_out = nc.declare_dram_parameter("g_out", [TOK_PER_CORE, NGROUPS], f16, isOutput=True)

    ctx = ExitStack()
    zt_all = ctx.enter_context(nc.sbuf_tensor("zt_all", [D2, TOK_PER_CORE], f16))
    wt_all = ctx.enter_context(nc.sbuf_tensor("wt_all", [D2, K], f16))
    xa = [
        ctx.enter_context(nc.sbuf_tensor(f"xa{i}", [128, K], f16)) for i in range(3)
    ]
    f1 = ctx.enter_context(nc.sbuf_tensor("f1", [128, K // 2], f16))
    gt = [
        ctx.enter_context(nc.sbuf_tensor(f"gt{i}", [128, NGROUPS], f16))
        for i in range(3)
    ]
    NPS = 4  # PSUM rotation depth: 4 duo-buffers x 2 banks = all 8 banks
    DUO = 2 * CHUNK  # 1024 fp32 = 2 banks
    ps = [
        ctx.enter_context(nc.psum_tensor(f"ps{i}", [128, DUO], f32))
        for i in range(NPS)
    ]

    s_load = ctx.enter_context(nc.semaphore("s_load"))
    s_mm = ctx.enter_context(nc.semaphore("s_mm"))
    s_act = ctx.enter_context(nc.semaphore("s_act"))
    s_chain = ctx.enter_context(nc.semaphore("s_chain"))
    s_f1 = ctx.enter_context(nc.semaphore("s_f1"))
    s_g = ctx.enter_context(nc.semaphore("s_g"))
    s_dma = ctx.enter_context(nc.semaphore("s_dma"))

    # duo d of each tile: 0-4 evacuated by ACT into xa; 5-7 (chunks 10-15)
    # chain-folded by DVE straight into f1 slots (the f1 pairing is
    # (chunk c, chunk c-8), distance 4096 positions = 2*NGROUPS: group-safe)
    ACT_DUOS = 5

    with nc.Block() as block:

        # duo ownership: ACT copies duos 0,1,2,3,5; DVE chain-folds duos
        # 4,6,7 from PSUM straight into f1 slots.  The f1 pairing is
        # (chunk c, chunk c+8): (0,8),(1,9) <- chain duo4; (2,10),(3,11)
        # <- fp16 merge of two ACT-written duos; (4,12),(5,13) <- duo6;
        # (6,14),(7,15) <- duo7.
        def pe_wait(eng, Dp):
            tp, dp = divmod(Dp, 8)
            if dp <= 3:
                eng.wait_ge(s_act, 5 * tp + dp + 1)
            elif dp == 5:
                eng.wait_ge(s_act, 5 * tp + 5)
            elif dp == 4:
                eng.wait_ge(s_chain, 3 * tp + 1)
            elif dp == 6:
                eng.wait_ge(s_chain, 3 * tp + 2)
            else:
                eng.wait_ge(s_chain, 3 * tp + 3)

        @block.tensor
        def _(eng):
            eng.wait_ge(s_load, 32)
            for t in range(T):
                ztile = zt_all[:, t * 128 : (t + 1) * 128]
                for d in range(8):
                    D = 8 * t + d
                    if D >= NPS:
                        pe_wait(eng, D - NPS)
                    for c2 in range(2):
                        c = d * 2 + c2
                        mm = nc.tensor.matmul(
                            ps[D % NPS][:, c2 * CHUNK : (c2 + 1) * CHUNK],
                            ztile,
                            wt_all[:, c * CHUNK : (c + 1) * CHUNK],
                            start=True,
                            stop=True,
                        )
                    mm.then_inc(s_mm, 1)

        @block.scalar
        def _(eng):
            for t in range(T):
                x = xa[t % 3]
                if t >= 3:
                    eng.wait_ge(s_f1, t - 2)
                for d in (0, 1, 2, 3, 5):
                    D = 8 * t + d
                    eng.wait_ge(s_mm, D + 1)
                    nc.scalar.copy(
                        x[:, d * DUO : (d + 1) * DUO], ps[D % NPS][:]
                    ).then_inc(s_act, 1)

        @block.vector
        def _(eng):
            for t in range(T):
                x = xa[t % 3]
                # chain duo4: psum chunks 8,9 onto xa chunks 0,1 -> f1[0:1024]
                eng.wait_ge(s_mm, 8 * t + 5)
                eng.wait_ge(s_act, 5 * t + 1)
                nc.vector.tensor_max(
                    f1[:, :DUO], ps[(8 * t + 4) % NPS][:], x[:, :DUO]
                ).then_inc(s_chain, 1)
                # chain duo6: chunks 12,13 onto xa chunks 4,5 -> f1[2048:3072]
                eng.wait_ge(s_mm, 8 * t + 7)
                eng.wait_ge(s_act, 5 * t + 3)
                nc.vector.tensor_max(
                    f1[:, 2 * DUO : 3 * DUO], ps[(8 * t + 6) % NPS][:], x[:, 2 * DUO : 3 * DUO]
                ).then_inc(s_chain, 1)
                # chain duo7: chunks 14,15 onto xa chunks 6,7 -> f1[3072:4096]
                eng.wait_ge(s_mm, 8 * t + 8)
                eng.wait_ge(s_act, 5 * t + 4)
                nc.vector.tensor_max(
                    f1[:, 3 * DUO : 4 * DUO], ps[(8 * t + 7) % NPS][:], x[:, 3 * DUO : 4 * DUO]
                ).then_inc(s_chain, 1)
                # fp16 merge: (chunks 2,3) vs (chunks 10,11) -> f1[1024:2048]
                eng.wait_ge(s_act, 5 * t + 5)
                nc.vector.tensor_max(
                    f1[:, DUO : 2 * DUO], x[:, DUO : 2 * DUO], x[:, 5 * DUO : 6 * DUO]
                ).then_inc(s_f1, 1)
                if t >= 3:
                    eng.wait_ge(s_dma, 16 * (t - 2))
                nc.vector.tensor_max(
                    gt[t % 3][:], f1[:, :NGROUPS], f1[:, NGROUPS:]
                ).then_inc(s_g, 1)

        @block.sync
        def _(eng):
            eng.dma_start(zt_all[:], zT_in[:]).then_inc(s_load, 16)
            eng.dma_start(wt_all[:], wT_in[:]).then_inc(s_load, 16)
            for t in range(T):
                eng.wait_ge(s_g, t + 1)
                eng.dma_start(
                    g_out[t * 128 : (t + 1) * 128, :], gt[t % 3][:]
                ).then_inc(s_dma, 16)

    nc.finalize()
    ctx.close()
    return nc


def _build_nc_tile(act_chunks=ACT_CHUNKS):
    """Tile-scheduled variant (kept for A/B comparison)."""
    import concourse.bacc as bacc
    import concourse.mybir as mybir
    from concourse.tile import TileContext

    nc = bacc.Bacc()
    zT_in = nc.declare_dram_parameter(
        "zT", [D2, TOK_PER_CORE], mybir.dt.float16, isOutput=False
    )
    wT_in = nc.declare_dram_parameter("wT", [D2, K], mybir.dt.float16, isOutput=False)
    g_out = nc.declare_dram_parameter(
        "g_out", [TOK_PER_CORE, NGROUPS], mybir.dt.float16, isOutput=True
    )

    with TileContext(nc) as tc:
        with (
            tc.tile_pool(name="static", bufs=1) as stat,
            tc.tile_pool(name="xa", bufs=3) as xpool,
            tc.tile_pool(name="f1", bufs=2) as fpool1,
            tc.tile_pool(name="f2", bufs=2) as fpool2,
            tc.tile_pool(name="g", bufs=2) as gpool,
            tc.tile_pool(name="ps", bufs=2, space="PSUM") as pspool,
        ):
            zt_all = stat.tile([D2, TOK_PER_CORE], mybir.dt.float16)
            wt_all = stat.tile([D2, K], mybir.dt.float16)
            nc.sync.dma_start(zt_all[:], zT_in[:])
            nc.sync.dma_start(wt_all[:], wT_in[:])

            for t in range(TILES_PER_CORE):
                ztile = zt_all[:, t * 128 : (t + 1) * 128]
                xa = xpool.tile([128, K], mybir.dt.float16)
                for quad in range(NCHUNKS // QUAD):
                    pq = pspool.tile([128, QUAD * CHUNK], mybir.dt.float32)
                    for c4 in range(QUAD):
                        c = quad * QUAD + c4
                        nc.tensor.matmul(
                            pq[:, c4 * CHUNK : (c4 + 1) * CHUNK],
                            ztile,
                            wt_all[:, c * CHUNK : (c + 1) * CHUNK],
                            start=True,
                            stop=True,
                        )
                    q0 = quad * QUAD  # first chunk id of the quad
                    if q0 + QUAD <= act_chunks:
                        nc.scalar.copy(
                            xa[:, q0 * CHUNK : (q0 + QUAD) * CHUNK], pq[:]
                        )
                    else:
                        # ACT takes chunks < act_chunks; DVE chain-folds the rest:
                        # xa[c] = max(psum_c, xa[c-1])
                        if q0 < act_chunks:
                            nc.scalar.copy(
                                xa[:, q0 * CHUNK : act_chunks * CHUNK],
                                pq[:, : (act_chunks - q0) * CHUNK],
                            )
                        # merge distance must be a multiple of NGROUPS positions
                        # (2 chunks) to keep group classes aligned
                        step = NGROUPS // CHUNK  # chunks per group period
                        for c in range(max(q0, act_chunks), q0 + QUAD):
                            nc.vector.tensor_max(
                                xa[:, c * CHUNK : (c + 1) * CHUNK],
                                pq[:, (c - q0) * CHUNK : (c - q0 + 1) * CHUNK],
                                xa[:, (c - step) * CHUNK : (c - step + 1) * CHUNK],
                            )
                f1 = fpool1.tile([128, K // 2], mybir.dt.float16)
                nc.vector.tensor_max(f1[:], xa[:, : K // 2], xa[:, K // 2 :])
                f2 = fpool2.tile([128, K // 4], mybir.dt.float16)
                nc.vector.tensor_max(f2[:], f1[:, : K // 4], f1[:, K // 4 :])
                g = gpool.tile([128, NGROUPS], mybir.dt.float16)
                nc.vector.tensor_max(g[:], f2[:, :NGROUPS], f2[:, NGROUPS:])
                nc.sync.dma_start(g_out[t * 128 : (t + 1) * 128, :], g[:])

    nc.finalize()
    return nc


def _get_nc():
    if "nc" not in _cached:
        _cached["nc"] = _build_nc()
    return _cached["nc"]


def _host_prep(z_real, z_imag, weight):
    z = np.concatenate(
        [np.asarray(z_real, np.float32), np.asarray(z_imag, np.float32)], axis=1
    )  # [N, 128]
    w = np.asarray(weight, np.float32)
    y64 = (w.astype(np.float64) ** 2).sum(1)  # [K]
    order = np.argsort(y64, kind="stable")
    ws = w[order]  # [K, 128] sorted by ||w||^2
    ys = y64[order]

    u = np.arange(K)
    slots = (u % GROUP) * NGROUPS + (u // GROUP)  # code u -> score slot
    wT_dev = np.empty((D2, K), np.float16)
    wT_dev[:, slots] = ws.T.astype(np.float16)
    zT = np.ascontiguousarray(z.T).astype(np.float16)  # [128, N]
    return z, w, order, ws, ys, wT_dev, zT


def _device_groupmax(nc, zT, wT_dev):
    from concourse.bass_utils import run_bass_kernel_spmd

    in_maps = [
        {
            "zT": np.ascontiguousarray(
                zT[:, c * TOK_PER_CORE : (c + 1) * TOK_PER_CORE]
            ),
            "wT": wT_dev,
        }
        for c in range(N_CORES)
    ]
    res = run_bass_kernel_spmd(nc, in_maps, list(range(N_CORES)))
    G = np.concatenate([r["g_out"] for r in res.results], axis=0)
    return G.astype(np.float32)  # [N, 512]


def _host_select(z, order, ws, ys, G):
    """Filter candidate groups per token and rescore exactly in fp64."""
    n = z.shape[0]
    y_grp = ys.reshape(NGROUPS, GROUP)
    y_lo = y_grp[:, 0]  # sorted ascending within group
    y_hi = y_grp[:, -1]

    U = G + MARGIN - (y_lo * 0.5)[None, :]
    L = G - MARGIN - (y_hi * 0.5)[None, :]
    bestL = L.max(axis=1)
    cand = U >= bestL[:, None]

    tok_idx, grp_idx = np.nonzero(cand)
    o = np.argsort(grp_idx, kind="stable")
    tok_o = tok_idx[o]
    grp_o = grp_idx[o]
    bounds = np.searchsorted(grp_o, np.arange(NGROUPS + 1))

    zf = z.astype(np.float64)
    wsf = ws.astype(np.float64)
    npairs = tok_o.shape[0]
    pair_val = np.empty(npairs, np.float64)
    pair_j = np.empty(npairs, np.int64)

    for g in range(NGROUPS):
        s0, s1 = bounds[g], bounds[g + 1]
        if s0 == s1:
            continue
        toks = tok_o[s0:s1]
        lo, hi = GROUP * g, GROUP * (g + 1)
        sc = zf[toks] @ wsf[lo:hi].T - 0.5 * ys[lo:hi]
        mx = sc.max(axis=1)
        orig = order[lo:hi]
        tied = sc == mx[:, None]
        jj = np.where(tied, orig[None, :], np.iinfo(np.int64).max).min(axis=1)
        pair_val[s0:s1] = mx
        pair_j[s0:s1] = jj

    best_val = np.full(n, -np.inf)
    np.maximum.at(best_val, tok_o, pair_val)
    is_best = pair_val == best_val[tok_o]
    best_j = np.full(n, np.iinfo(np.int64).max)
    np.minimum.at(best_j, tok_o[is_best], pair_j[is_best])
    return best_j.astype(np.int64)


def kernel(z_real, z_imag, weight):
    z, w, order, ws, ys, wT_dev, zT = _host_prep(z_real, z_imag, weight)
    nc = _get_nc()
    G = _device_groupmax(nc, zT, wT_dev)
    idx = _host_select(z, order, ws, ys, G)

    z_q = w[idx]  # [N, 128] fp32
    z_q_c = (z_q[:, :DIM] + 1j * z_q[:, DIM:]).astype(np.complex64)
    diff = z_q.astype(np.float64) - z.astype(np.float64)
    loss_sample = (1.25 * (diff**2).mean(axis=1)).astype(np.float32)
    indices = idx.astype(np.int32)
    counts = np.bincount(idx, minlength=K).astype(np.float64)
    avg_probs = counts / z.shape[0]
    batch_entropy = np.float32(-(avg_probs * np.log(avg_probs + 1e-10)).sum())
    return z_q_c, loss_sample, indices, batch_entropy


# revision 21
# speedup vs baseline: 1.0945x; 1.0004x over previous
"""Trainium2 kernel for EntropyRegularizedVQ (vq_codebook).

Contract: kernel(z_real, z_imag, weight) -> (z_q_c, loss_sample, indices, batch_entropy)
matching reference.py numerics. Self-contained: hardcoded shapes/sharding.

Strategy
--------
Device (8 NeuronCores, data-parallel over tokens, codebook replicated):
  scores[t, j] = z_t . w_j computed as fp16 matmul (fp32 PSUM accumulate).
  The codebook is pre-sorted by ||w||^2 and laid out so that a contiguous
  binary max-fold of the 8192 score slots yields, per token, the max score
  of each GROUP of 16 codes with adjacent ||w||^2.  ACT evacuates PSUM
  (fp32 -> fp16), DVE does the 4-level fold, DMA returns G [tokens, 512].

Host:
  A rigorous interval filter (fp16 ulp + matmul-error margin + per-group
  ||w||^2 range) selects ~1.1 candidate groups per token; those 16-code
  groups are rescored exactly in fp64 to get the argmin index.  All other
  outputs (z_q, loss, histogram entropy) are cheap host math.
"""

import sys

for _p in ("/opt/trn_rl_repo", "/root/.axon_site/_ro/trn_rl_repo"):
    if _p not in sys.path:
        sys.path.insert(0, _p)

import numpy as np

N_TOKENS = 32768
DIM = 64
D2 = 2 * DIM  # 128
K = 8192
N_CORES = 8
TOK_PER_CORE = N_TOKENS // N_CORES  # 4096
TILES_PER_CORE = TOK_PER_CORE // 128  # 32
GROUP = 4  # codes per group (adjacent in ||w||^2)
NGROUPS = K // GROUP  # 2048
CHUNK = 512  # matmul moving free dim (one PSUM bank of fp32)
NCHUNKS = K // CHUNK  # 16
QUAD = 4  # chunks per PSUM tile (4 banks)
ACT_CHUNKS = 12  # chunks evacuated by ACT; the rest are DVE chain-folded

# margin (score scale): 8-sigma fp16 matmul error + fp16 ulp at |s|<=16
MARGIN = 0.015

_cached = {}


def _build_nc(act_chunks=ACT_CHUNKS):
    """Hand-scheduled raw-bass SPMD program (one core's view).

    Engine streams (per 128-token tile):
      PE:   4 quads x 4 matmuls (N=512) -> 2 ping-pong PSUM tiles [128,2048]
      ACT:  quads 0-2: copy PSUM fp32 -> xa fp16 (evacuation)
      DVE:  quad 3: chain-folds max(psum, xa[c-2]) into xa slots 12-15,
            then 3 binary folds xa[8192] -> G[1024], signal DMA
      SYNC: g_out DMA per tile
    """
    from contextlib import ExitStack

    import concourse.bacc as bacc
    import concourse.mybir as mybir

    T = TILES_PER_CORE
    f16 = mybir.dt.float16
    f32 = mybir.dt.float32

    nc = bacc.Bacc()
    zT_in = nc.declare_dram_parameter("zT", [D2, TOK_PER_CORE], f16, isOutput=False)
    wT_in = nc.declare_dram_parameter("wT", [D2, K], f16, isOutput=False)
    g_out = nc.declare_dram_parameter("g_out", [TOK_PER_CORE, NGROUPS], f16, isOutput=True)

    ctx = ExitStack()
    zt_all = ctx.enter_context(nc.sbuf_tensor("zt_all", [D2, TOK_PER_CORE], f16))
    wt_all = ctx.enter_context(nc.sbuf_tensor("wt_all", [D2, K], f16))
    xa = [
        ctx.enter_context(nc.sbuf_tensor(f"xa{i}", [128, K], f16)) for i in range(3)
    ]
    f1 = ctx.enter_context(nc.sbuf_tensor("f1", [128, K // 2], f16))
    gt = [
        ctx.enter_context(nc.sbuf_tensor(f"gt{i}", [128, NGROUPS], f16))
        for i in range(3)
    ]
    NPS = 4  # PSUM rotation depth: 4 duo-buffers x 2 banks = all 8 banks
    DUO = 2 * CHUNK  # 1024 fp32 = 2 banks
    ps = [
        ctx.enter_context(nc.psum_tensor(f"ps{i}", [128, DUO], f32))
        for i in range(NPS)
    ]

    s_load = ctx.enter_context(nc.semaphore("s_load"))
    s_mm = ctx.enter_context(nc.semaphore("s_mm"))
    s_act = ctx.enter_context(nc.semaphore("s_act"))
    s_chain = ctx.enter_context(nc.semaphore("s_chain"))
    s_f1 = ctx.enter_context(nc.semaphore("s_f1"))
    s_g = ctx.enter_context(nc.semaphore("s_g"))
    s_dma = ctx.enter_context(nc.semaphore("s_dma"))

    # duo d of each tile: 0-4 evacuated by ACT into xa; 5-7 (chunks 10-15)
    # chain-folded by DVE straight into f1 slots (the f1 pairing is
    # (chunk c, chunk c-8), distance 4096 positions = 2*NGROUPS: group-safe)
    ACT_DUOS = 5

    with nc.Block() as block:

        # duo ownership: ACT copies duos 0,1,2,3,5; DVE chain-folds duos
        # 4,6,7 from PSUM straight into f1 slots.  The f1 pairing is
        # (chunk c, chunk c+8): (0,8),(1,9) <- chain duo4; (2,10),(3,11)
        # <- fp16 merge of two ACT-written duos; (4,12),(5,13) <- duo6;
        # (6,14),(7,15) <- duo7.
        def pe_wait(eng, Dp):
            tp, dp = divmod(Dp, 8)
            if dp <= 3:
                eng.wait_ge(s_act, 5 * tp + dp + 1)
            elif dp == 5:
                eng.wait_ge(s_act, 5 * tp + 5)
            elif dp == 4:
                eng.wait_ge(s_chain, 3 * tp + 1)
            elif dp == 6:
                eng.wait_ge(s_chain, 3 * tp + 2)
            else:
                eng.wait_ge(s_chain, 3 * tp + 3)

        @block.tensor
        def _(eng):
            eng.wait_ge(s_load, 32)
            for t in range(T):
                ztile = zt_all[:, t * 128 : (t + 1) * 128]
                for d in range(8):
                    D = 8 * t + d
                    if D >= NPS:
                        pe_wait(eng, D - NPS)
                    for c2 in range(2):
                        c = d * 2 + c2
                        mm = nc.tensor.matmul(
                            ps[D % NPS][:, c2 * CHUNK : (c2 + 1) * CHUNK],
                            ztile,
                            wt_all[:, c * CHUNK : (c + 1) * CHUNK],
                            start=True,
                            stop=True,
                        )
                    mm.then_inc(s_mm, 1)

        @block.scalar
        def _(eng):
            for t in range(T):
                x = xa[t % 3]
                if t >= 3:
                    eng.wait_ge(s_f1, t - 2)
                for d in (0, 1, 2, 3, 5):
                    D = 8 * t + d
                    eng.wait_ge(s_mm, D + 1)
                    nc.scalar.copy(
                        x[:, d * DUO : (d + 1) * DUO], ps[D % NPS][:]
                    ).then_inc(s_act, 1)

        @block.vector
        def _(eng):
            for t in range(T):
                x = xa[t % 3]
                # chain duo4: psum chunks 8,9 onto xa chunks 0,1 -> f1[0:1024]
                eng.wait_ge(s_mm, 8 * t + 5)
                eng.wait_ge(s_act, 5 * t + 1)
                nc.vector.tensor_max(
                    f1[:, :DUO], ps[(8 * t + 4) % NPS][:], x[:, :DUO]
                ).then_inc(s_chain, 1)
                # chain duo6: chunks 12,13 onto xa chunks 4,5 -> f1[2048:3072]
                eng.wait_ge(s_mm, 8 * t + 7)
                eng.wait_ge(s_act, 5 * t + 3)
                nc.vector.tensor_max(
                    f1[:, 2 * DUO : 3 * DUO], ps[(8 * t + 6) % NPS][:], x[:, 2 * DUO : 3 * DUO]
                ).then_inc(s_chain, 1)
                # chain duo7: chunks 14,15 onto xa chunks 6,7 -> f1[3072:4096]
                eng.wait_ge(s_mm, 8 * t + 8)
                eng.wait_ge(s_act, 5 * t + 4)
                nc.vector.tensor_max(
                    f1[:, 3 * DUO : 4 * DUO], ps[(8 * t + 7) % NPS][:], x[:, 3 * DUO : 4 * DUO]
                ).then_inc(s_chain, 1)
                # fp16 merge: (chunks 2,3) vs (chunks 10,11) -> f1[1024:2048]
                eng.wait_ge(s_act, 5 * t + 5)
                nc.vector.tensor_max(
                    f1[:, DUO : 2 * DUO], x[:, DUO : 2 * DUO], x[:, 5 * DUO : 6 * DUO]
                ).then_inc(s_f1, 1)
                if t >= 3:
                    eng.wait_ge(s_dma, 16 * (t - 2))
                nc.vector.tensor_max(
                    gt[t % 3][:], f1[:, :NGROUPS], f1[:, NGROUPS:]
                ).then_inc(s_g, 1)

        @block.sync
        def _(eng):
            eng.dma_start(zt_all[:], zT_in[:]).then_inc(s_load, 16)
            eng.dma_start(wt_all[:], wT_in[:]).then_inc(s_load, 16)
            for t in range(T):
                eng.wait_ge(s_g, t + 1)
                eng.dma_start(
                    g_out[t * 128 : (t + 1) * 128, :], gt[t % 3][:]
                ).then_inc(s_dma, 16)

    nc.finalize()
    ctx.close()
    return nc


def _build_nc_tile(act_chunks=ACT_CHUNKS):
    """Tile-scheduled variant (kept for A/B comparison)."""
    import concourse.bacc as bacc
    import concourse.mybir as mybir
    from concourse.tile import TileContext

    nc = bacc.Bacc()
    zT_in = nc.declare_dram_parameter(
        "zT", [D2, TOK_PER_CORE], mybir.dt.float16, isOutput=False
    )
    wT_in = nc.declare_dram_parameter("wT", [D2, K], mybir.dt.float16, isOutput=False)
    g_out = nc.declare_dram_parameter(
        "g_out", [TOK_PER_CORE, NGROUPS], mybir.dt.float16, isOutput=True
    )

    with TileContext(nc) as tc:
        with (
            tc.tile_pool(name="static", bufs=1) as stat,
            tc.tile_pool(name="xa", bufs=3) as xpool,
            tc.tile_pool(name="f1", bufs=2) as fpool1,
            tc.tile_pool(name="f2", bufs=2) as fpool2,
            tc.tile_pool(name="g", bufs=2) as gpool,
            tc.tile_pool(name="ps", bufs=2, space="PSUM") as pspool,
        ):
            zt_all = stat.tile([D2, TOK_PER_CORE], mybir.dt.float16)
            wt_all = stat.tile([D2, K], mybir.dt.float16)
            nc.sync.dma_start(zt_all[:], zT_in[:])
            nc.sync.dma_start(wt_all[:], wT_in[:])

            for t in range(TILES_PER_CORE):
                ztile = zt_all[:, t * 128 : (t + 1) * 128]
                xa = xpool.tile([128, K], mybir.dt.float16)
                for quad in range(NCHUNKS // QUAD):
                    pq = pspool.tile([128, QUAD * CHUNK], mybir.dt.float32)
                    for c4 in range(QUAD):
                        c = quad * QUAD + c4
                        nc.tensor.matmul(
                            pq[:, c4 * CHUNK : (c4 + 1) * CHUNK],
                            ztile,
                            wt_all[:, c * CHUNK : (c + 1) * CHUNK],
                            start=True,
                            stop=True,
                        )
                    q0 = quad * QUAD  # first chunk id of the quad
                    if q0 + QUAD <= act_chunks:
                        nc.scalar.copy(
                            xa[:, q0 * CHUNK : (q0 + QUAD) * CHUNK], pq[:]
                        )
                    else:
                        # ACT takes chunks < act_chunks; DVE chain-folds the rest:
                        # xa[c] = max(psum_c, xa[c-1])
                        if q0 < act_chunks:
                            nc.scalar.copy(
                                xa[:, q0 * CHUNK : act_chunks * CHUNK],
                                pq[:, : (act_chunks - q0) * CHUNK],
                            )
                        # merge distance must be a multiple of NGROUPS positions
                        # (2 chunks) to keep group classes aligned
                        step = NGROUPS // CHUNK  # chunks per group period
                        for c in range(max(q0, act_chunks), q0 + QUAD):
                            nc.vector.tensor_max(
                                xa[:, c * CHUNK : (c + 1) * CHUNK],
                                pq[:, (c - q0) * CHUNK : (c - q0 + 1) * CHUNK],
                                xa[:, (c - step) * CHUNK : (c - step + 1) * CHUNK],
                            )
                f1 = fpool1.tile([128, K // 2], mybir.dt.float16)
                nc.vector.tensor_max(f1[:], xa[:, : K // 2], xa[:, K // 2 :])
                f2 = fpool2.tile([128, K // 4], mybir.dt.float16)
                nc.vector.tensor_max(f2[:], f1[:, : K // 4], f1[:, K // 4 :])
                g = gpool.tile([128, NGROUPS], mybir.dt.float16)
                nc.vector.tensor_max(g[:], f2[:, :NGROUPS], f2[:, NGROUPS:])
                nc.sync.dma_start(g_out[t * 128 : (t + 1) * 128, :], g[:])

    nc.finalize()
    return nc


def _get_nc():
    if "nc" not in _cached:
        _cached["nc"] = _build_nc()
    return _cached["nc"]


def _host_prep(z_real, z_imag, weight):
    z = np.concatenate(
        [np.asarray(z_real, np.float32), np.asarray(z_imag, np.float32)], axis=1
    )  # [N, 128]
    w = np.asarray(weight, np.float32)
    y64 = (w.astype(np.float64) ** 2).sum(1)  # [K]
    order = np.argsort(y64, kind="stable")
    ws = w[order]  # [K, 128] sorted by ||w||^2
    ys = y64[order]

    u = np.arange(K)
    slots = (u % GROUP) * NGROUPS + (u // GROUP)  # code u -> score slot
    wT_dev = np.empty((D2, K), np.float16)
    wT_dev[:, slots] = ws.T.astype(np.float16)
    zT = np.ascontiguousarray(z.T).astype(np.float16)  # [128, N]
    return z, w, order, ws, ys, wT_dev, zT


def _device_groupmax(nc, zT, wT_dev):
    from concourse.bass_utils import run_bass_kernel_spmd

    in_maps = [
        {
            "zT": np.ascontiguousarray(
                zT[:, c * TOK_PER_CORE : (c + 1) * TOK_PER_CORE]
            ),
            "wT": wT_dev,
        }
        for c in range(N_CORES)
    ]
    res = run_bass_kernel_spmd(nc, in_maps, list(range(N_CORES)))
    G = np.concatenate([r["g_out"] for r in res.results], axis=0)
    return G.astype(np.float32)  # [N, 512]


def _host_select(z, order, ws, ys, G):
    """Filter candidate groups per token and rescore exactly in fp64."""
    n = z.shape[0]
    y_grp = ys.reshape(NGROUPS, GROUP)
    y_lo = y_grp[:, 0]  # sorted ascending within group
    y_hi = y_grp[:, -1]

    U = G + MARGIN - (y_lo * 0.5)[None, :]
    L = G - MARGIN - (y_hi * 0.5)[None, :]
    bestL = L.max(axis=1)
    cand = U >= bestL[:, None]

    tok_o, grp_o = np.nonzero(cand)  # ~1.1 pairs per token

    zf = z.astype(np.float64)
    wsf = ws.astype(np.float64)
    w_blk = wsf.reshape(NGROUPS, GROUP, D2)[grp_o]  # [P, GROUP, 128]
    sc = np.matmul(w_blk, zf[tok_o][:, :, None])[:, :, 0]  # [P, GROUP]
    sc -= 0.5 * ys.reshape(NGROUPS, GROUP)[grp_o]
    mx = sc.max(axis=1)
    orig = order.reshape(NGROUPS, GROUP)[grp_o]  # original code ids
    tied = sc == mx[:, None]
    jj = np.where(tied, orig, np.iinfo(np.int64).max).min(axis=1)

    best_val = np.full(n, -np.inf)
    np.maximum.at(best_val, tok_o, mx)
    is_best = mx == best_val[tok_o]
    best_j = np.full(n, np.iinfo(np.int64).max)
    np.minimum.at(best_j, tok_o[is_best], jj[is_best])
    return best_j.astype(np.int64)


def kernel(z_real, z_imag, weight):
    z, w, order, ws, ys, wT_dev, zT = _host_prep(z_real, z_imag, weight)
    nc = _get_nc()
    G = _device_groupmax(nc, zT, wT_dev)
    idx = _host_select(z, order, ws, ys, G)

    z_q = w[idx]  # [N, 128] fp32
    z_q_c = (z_q[:, :DIM] + 1j * z_q[:, DIM:]).astype(np.complex64)
    diff = z_q.astype(np.float64) - z.astype(np.float64)
    loss_sample = (1.25 * (diff**2).mean(axis=1)).astype(np.float32)
    indices = idx.astype(np.int32)
    counts = np.bincount(idx, minlength=K).astype(np.float64)
    avg_probs = counts / z.shape[0]
    batch_entropy = np.float32(-(avg_probs * np.log(avg_probs + 1e-10)).sum())
    return z_q_c, loss_sample, indices, batch_entropy


# revision 22
# speedup vs baseline: 1.0950x; 1.0004x over previous
"""Trainium2 kernel for EntropyRegularizedVQ (vq_codebook).

Contract: kernel(z_real, z_imag, weight) -> (z_q_c, loss_sample, indices, batch_entropy)
matching reference.py numerics. Self-contained: hardcoded shapes/sharding.

Strategy
--------
Device (8 NeuronCores, data-parallel over tokens, codebook replicated):
  scores[t, j] = z_t . w_j computed as fp16 matmul (fp32 PSUM accumulate).
  The codebook is pre-sorted by ||w||^2 and laid out so that a contiguous
  binary max-fold of the 8192 score slots yields, per token, the max score
  of each GROUP of 4 codes with adjacent ||w||^2.  ACT evacuates PSUM
  (fp32 -> fp16), DVE folds, DMA returns G [tokens, 2048] fp16.

Host:
  A rigorous interval filter (fp16 ulp + matmul-error margin + per-group
  ||w||^2 range) selects ~1.1 candidate groups per token; those 4-code
  groups are rescored exactly in fp64 to get the argmin index.  All other
  outputs (z_q, loss, histogram entropy) are cheap host math.
"""

import sys

for _p in ("/opt/trn_rl_repo", "/root/.axon_site/_ro/trn_rl_repo"):
    if _p not in sys.path:
        sys.path.insert(0, _p)

import numpy as np

N_TOKENS = 32768
DIM = 64
D2 = 2 * DIM  # 128
K = 8192
N_CORES = 8
TOK_PER_CORE = N_TOKENS // N_CORES  # 4096
TILES_PER_CORE = TOK_PER_CORE // 128  # 32
GROUP = 4  # codes per group (adjacent in ||w||^2)
NGROUPS = K // GROUP  # 2048
CHUNK = 512  # matmul moving free dim (one PSUM bank of fp32)
NCHUNKS = K // CHUNK  # 16
QUAD = 4  # chunks per PSUM tile (4 banks)

# margin (score scale) bounding |fp16(s_dev) - s_fp32|:
#   fp16 matmul input-rounding error: sigma ~ 6.1e-5*|z|, |z| <= ~14.5
#   over 32768 tokens -> 16-sigma ~ 0.014; plus fp16 output rounding
#   ulp/2 <= 0.0078 for |s| < 32.  0.022 covers both with headroom.
MARGIN = 0.022

_cached = {}


def _build_nc():
    """Hand-scheduled raw-bass SPMD program (one core's view).

    Engine streams (per 128-token tile; duo = 2 matmul chunks = 1024 slots):
      PE:   8 duos x 2 matmuls (N=512) -> 4 rotating PSUM tiles [128,1024]
      ACT:  duos 0,1,2,3,5: copy PSUM fp32 -> xa fp16 (evacuation)
      DVE:  duos 4,6,7: chain-fold max(PSUM, xa-partner) straight into f1
            slots; one fp16 merge for the (2,3)x(10,11) pair; final fold
            f1[4096] -> G[2048]; signal DMA
      SYNC: g_out DMA per tile
    """
    from contextlib import ExitStack

    import concourse.bacc as bacc
    import concourse.mybir as mybir

    T = TILES_PER_CORE
    f16 = mybir.dt.float16
    f32 = mybir.dt.float32

    nc = bacc.Bacc()
    zT_in = nc.declare_dram_parameter("zT", [D2, TOK_PER_CORE], f16, isOutput=False)
    wT_in = nc.declare_dram_parameter("wT", [D2, K], f16, isOutput=False)
    g_out = nc.declare_dram_parameter("g_out", [TOK_PER_CORE, NGROUPS], f16, isOutput=True)

    ctx = ExitStack()
    zt_all = ctx.enter_context(nc.sbuf_tensor("zt_all", [D2, TOK_PER_CORE], f16))
    wt_all = ctx.enter_context(nc.sbuf_tensor("wt_all", [D2, K], f16))
    xa = [
        ctx.enter_context(nc.sbuf_tensor(f"xa{i}", [128, K], f16)) for i in range(3)
    ]
    f1 = ctx.enter_context(nc.sbuf_tensor("f1", [128, K // 2], f16))
    gt = [
        ctx.enter_context(nc.sbuf_tensor(f"gt{i}", [128, NGROUPS], f16))
        for i in range(3)
    ]
    NPS = 4  # PSUM rotation depth: 4 duo-buffers x 2 banks = all 8 banks
    DUO = 2 * CHUNK  # 1024 fp32 = 2 banks
    ps = [
        ctx.enter_context(nc.psum_tensor(f"ps{i}", [128, DUO], f32))
        for i in range(NPS)
    ]

    s_load = ctx.enter_context(nc.semaphore("s_load"))
    s_mm = ctx.enter_context(nc.semaphore("s_mm"))
    s_act = ctx.enter_context(nc.semaphore("s_act"))
    s_chain = ctx.enter_context(nc.semaphore("s_chain"))
    s_f1 = ctx.enter_context(nc.semaphore("s_f1"))
    s_g = ctx.enter_context(nc.semaphore("s_g"))
    s_dma = ctx.enter_context(nc.semaphore("s_dma"))

    with nc.Block() as block:

        # duo ownership: ACT copies duos 0,1,2,3,5; DVE chain-folds duos
        # 4,6,7 from PSUM straight into f1 slots.  The f1 pairing is
        # (chunk c, chunk c+8): (0,8),(1,9) <- chain duo4; (2,10),(3,11)
        # <- fp16 merge of two ACT-written duos; (4,12),(5,13) <- duo6;
        # (6,14),(7,15) <- duo7.
        def pe_wait(eng, Dp):
            tp, dp = divmod(Dp, 8)
            if dp <= 3:
                eng.wait_ge(s_act, 5 * tp + dp + 1)
            elif dp == 5:
                eng.wait_ge(s_act, 5 * tp + 5)
            elif dp == 4:
                eng.wait_ge(s_chain, 3 * tp + 1)
            elif dp == 6:
                eng.wait_ge(s_chain, 3 * tp + 2)
            else:
                eng.wait_ge(s_chain, 3 * tp + 3)

        @block.tensor
        def _(eng):
            eng.wait_ge(s_load, 32)
            for t in range(T):
                ztile = zt_all[:, t * 128 : (t + 1) * 128]
                for d in range(8):
                    D = 8 * t + d
                    if D >= NPS:
                        pe_wait(eng, D - NPS)
                    for c2 in range(2):
                        c = d * 2 + c2
                        mm = nc.tensor.matmul(
                            ps[D % NPS][:, c2 * CHUNK : (c2 + 1) * CHUNK],
                            ztile,
                            wt_all[:, c * CHUNK : (c + 1) * CHUNK],
                            start=True,
                            stop=True,
                        )
                    mm.then_inc(s_mm, 1)

        @block.scalar
        def _(eng):
            for t in range(T):
                x = xa[t % 3]
                if t >= 3:
                    eng.wait_ge(s_f1, t - 2)
                for d in (0, 1, 2, 3, 5):
                    D = 8 * t + d
                    eng.wait_ge(s_mm, D + 1)
                    nc.scalar.copy(
                        x[:, d * DUO : (d + 1) * DUO], ps[D % NPS][:]
                    ).then_inc(s_act, 1)

        @block.vector
        def _(eng):
            for t in range(T):
                x = xa[t % 3]
                # chain duo4: psum chunks 8,9 onto xa chunks 0,1 -> f1[0:1024]
                eng.wait_ge(s_mm, 8 * t + 5)
                eng.wait_ge(s_act, 5 * t + 1)
                nc.vector.tensor_max(
                    f1[:, :DUO], ps[(8 * t + 4) % NPS][:], x[:, :DUO]
                ).then_inc(s_chain, 1)
                # chain duo6: chunks 12,13 onto xa chunks 4,5 -> f1[2048:3072]
                eng.wait_ge(s_mm, 8 * t + 7)
                eng.wait_ge(s_act, 5 * t + 3)
                nc.vector.tensor_max(
                    f1[:, 2 * DUO : 3 * DUO], ps[(8 * t + 6) % NPS][:], x[:, 2 * DUO : 3 * DUO]
                ).then_inc(s_chain, 1)
                # chain duo7: chunks 14,15 onto xa chunks 6,7 -> f1[3072:4096]
                eng.wait_ge(s_mm, 8 * t + 8)
                eng.wait_ge(s_act, 5 * t + 4)
                nc.vector.tensor_max(
                    f1[:, 3 * DUO : 4 * DUO], ps[(8 * t + 7) % NPS][:], x[:, 3 * DUO : 4 * DUO]
                ).then_inc(s_chain, 1)
                # fp16 merge: (chunks 2,3) vs (chunks 10,11) -> f1[1024:2048]
                eng.wait_ge(s_act, 5 * t + 5)
                nc.vector.tensor_max(
                    f1[:, DUO : 2 * DUO], x[:, DUO : 2 * DUO], x[:, 5 * DUO : 6 * DUO]
                ).then_inc(s_f1, 1)
                if t >= 3:
                    eng.wait_ge(s_dma, 16 * (t - 2))
                nc.vector.tensor_max(
                    gt[t % 3][:], f1[:, :NGROUPS], f1[:, NGROUPS:]
                ).then_inc(s_g, 1)

        @block.sync
        def _(eng):
            eng.dma_start(zt_all[:], zT_in[:]).then_inc(s_load, 16)
            eng.dma_start(wt_all[:], wT_in[:]).then_inc(s_load, 16)
            for t in range(T):
                eng.wait_ge(s_g, t + 1)
                eng.dma_start(
                    g_out[t * 128 : (t + 1) * 128, :], gt[t % 3][:]
                ).then_inc(s_dma, 16)

    nc.finalize()
    ctx.close()
    return nc


def _get_nc():
    if "nc" not in _cached:
        _cached["nc"] = _build_nc()
    return _cached["nc"]


def _host_prep(z_real, z_imag, weight):
    z = np.concatenate(
        [np.asarray(z_real, np.float32), np.asarray(z_imag, np.float32)], axis=1
    )  # [N, 128]
    w = np.asarray(weight, np.float32)
    y64 = (w.astype(np.float64) ** 2).sum(1)  # [K]
    order = np.argsort(y64, kind="stable")
    ws = w[order]  # [K, 128] sorted by ||w||^2
    ys = y64[order]

    u = np.arange(K)
    slots = (u % GROUP) * NGROUPS + (u // GROUP)  # code u -> score slot
    wT_dev = np.empty((D2, K), np.float16)
    wT_dev[:, slots] = ws.T.astype(np.float16)
    zT = np.ascontiguousarray(z.T).astype(np.float16)  # [128, N]
    return z, w, order, ws, ys, wT_dev, zT


def _device_groupmax(nc, zT, wT_dev):
    from concourse.bass_utils import run_bass_kernel_spmd

    in_maps = [
        {
            "zT": np.ascontiguousarray(
                zT[:, c * TOK_PER_CORE : (c + 1) * TOK_PER_CORE]
            ),
            "wT": wT_dev,
        }
        for c in range(N_CORES)
    ]
    res = run_bass_kernel_spmd(nc, in_maps, list(range(N_CORES)))
    G = np.concatenate([r["g_out"] for r in res.results], axis=0)
    return G.astype(np.float32)  # [N, 512]


def _host_select(z, order, ws, ys, G):
    """Filter candidate groups per token and rescore exactly in fp64."""
    n = z.shape[0]
    y_grp = ys.reshape(NGROUPS, GROUP)
    y_lo = y_grp[:, 0]  # sorted ascending within group
    y_hi = y_grp[:, -1]

    U = G + MARGIN - (y_lo * 0.5)[None, :]
    L = G - MARGIN - (y_hi * 0.5)[None, :]
    bestL = L.max(axis=1)
    cand = U >= bestL[:, None]

    tok_o, grp_o = np.nonzero(cand)  # ~1.1 pairs per token

    zf = z.astype(np.float64)
    wsf = ws.astype(np.float64)
    w_blk = wsf.reshape(NGROUPS, GROUP, D2)[grp_o]  # [P, GROUP, 128]
    sc = np.matmul(w_blk, zf[tok_o][:, :, None])[:, :, 0]  # [P, GROUP]
    sc -= 0.5 * ys.reshape(NGROUPS, GROUP)[grp_o]
    mx = sc.max(axis=1)
    orig = order.reshape(NGROUPS, GROUP)[grp_o]  # original code ids
    tied = sc == mx[:, None]
    jj = np.where(tied, orig, np.iinfo(np.int64).max).min(axis=1)

    best_val = np.full(n, -np.inf)
    np.maximum.at(best_val, tok_o, mx)
    is_best = mx == best_val[tok_o]
    best_j = np.full(n, np.iinfo(np.int64).max)
    np.minimum.at(best_j, tok_o[is_best], jj[is_best])
    return best_j.astype(np.int64)


def kernel(z_real, z_imag, weight):
    z, w, order, ws, ys, wT_dev, zT = _host_prep(z_real, z_imag, weight)
    nc = _get_nc()
    G = _device_groupmax(nc, zT, wT_dev)
    idx = _host_select(z, order, ws, ys, G)

    z_q = w[idx]  # [N, 128] fp32
    z_q_c = (z_q[:, :DIM] + 1j * z_q[:, DIM:]).astype(np.complex64)
    diff = z_q.astype(np.float64) - z.astype(np.float64)
    loss_sample = (1.25 * (diff**2).mean(axis=1)).astype(np.float32)
    indices = idx.astype(np.int32)
    counts = np.bincount(idx, minlength=K).astype(np.float64)
    avg_probs = counts / z.shape[0]
    batch_entropy = np.float32(-(avg_probs * np.log(avg_probs + 1e-10)).sum())
    return z_q_c, loss_sample, indices, batch_entropy


# revision 23
# speedup vs baseline: 1.1167x; 1.0198x over previous
"""Trainium2 kernel for EntropyRegularizedVQ (vq_codebook).

Contract: kernel(z_real, z_imag, weight) -> (z_q_c, loss_sample, indices, batch_entropy)
matching reference.py numerics. Self-contained: hardcoded shapes/sharding.

Strategy
--------
Device (8 NeuronCores, data-parallel over tokens, codebook replicated):
  scores[t, j] = z_t . w_j computed as fp16 matmul (fp32 PSUM accumulate).
  The codebook is pre-sorted by ||w||^2 and laid out so that a contiguous
  binary max-fold of the 8192 score slots yields, per token, the max score
  of each GROUP of 4 codes with adjacent ||w||^2.  ACT evacuates PSUM
  (fp32 -> fp16), DVE folds, DMA returns G [tokens, 2048] fp16.

Host:
  A rigorous interval filter (fp16 ulp + matmul-error margin + per-group
  ||w||^2 range) selects ~1.1 candidate groups per token; those 4-code
  groups are rescored exactly in fp64 to get the argmin index.  All other
  outputs (z_q, loss, histogram entropy) are cheap host math.
"""

import sys

for _p in ("/opt/trn_rl_repo", "/root/.axon_site/_ro/trn_rl_repo"):
    if _p not in sys.path:
        sys.path.insert(0, _p)

import numpy as np

N_TOKENS = 32768
DIM = 64
D2 = 2 * DIM  # 128
K = 8192
N_CORES = 8
TOK_PER_CORE = N_TOKENS // N_CORES  # 4096
TILES_PER_CORE = TOK_PER_CORE // 128  # 32
GROUP = 4  # codes per group (adjacent in ||w||^2)
NGROUPS = K // GROUP  # 2048
CHUNK = 512  # matmul moving free dim (one PSUM bank of fp32)
NCHUNKS = K // CHUNK  # 16
QUAD = 4  # chunks per PSUM tile (4 banks)

# margin (score scale) bounding |fp16(s_dev) - s_fp32|:
#   fp16 matmul input-rounding error: sigma ~ 6.1e-5*|z|, |z| <= ~14.5
#   over 32768 tokens -> 16-sigma ~ 0.014; plus fp16 output rounding
#   ulp/2 <= 0.0078 for |s| < 32.  0.022 covers both with headroom.
MARGIN = 0.022

_cached = {}


def _build_nc():
    """Hand-scheduled raw-bass SPMD program (one core's view).

    Engine streams (per 128-token tile; duo = 2 matmul chunks = 1024 slots):
      PE:   8 duos x 2 matmuls (N=512) -> 4 rotating PSUM tiles [128,1024]
      ACT:  duos 0,1,2,3,5: copy PSUM fp32 -> xa fp16 (evacuation)
      DVE:  duos 4,6,7: chain-fold max(PSUM, xa-partner) straight into f1
            slots; one fp16 merge for the (2,3)x(10,11) pair; final fold
            f1[4096] -> G[2048]; signal DMA
      SYNC: g_out DMA per tile
    """
    from contextlib import ExitStack

    import concourse.bacc as bacc
    import concourse.mybir as mybir

    T = TILES_PER_CORE
    f16 = mybir.dt.float16
    f32 = mybir.dt.float32

    nc = bacc.Bacc()
    zT_in = nc.declare_dram_parameter("zT", [D2, TOK_PER_CORE], f16, isOutput=False)
    wT_in = nc.declare_dram_parameter("wT", [D2, K], f16, isOutput=False)
    g_out = nc.declare_dram_parameter("g_out", [TOK_PER_CORE, NGROUPS], f16, isOutput=True)

    ctx = ExitStack()
    zt_all = ctx.enter_context(nc.sbuf_tensor("zt_all", [D2, TOK_PER_CORE], f16))
    wt_all = ctx.enter_context(nc.sbuf_tensor("wt_all", [D2, K], f16))
    xa = [
        ctx.enter_context(nc.sbuf_tensor(f"xa{i}", [128, K], f16)) for i in range(3)
    ]
    f1 = ctx.enter_context(nc.sbuf_tensor("f1", [128, K // 2], f16))
    gt = [
        ctx.enter_context(nc.sbuf_tensor(f"gt{i}", [128, NGROUPS], f16))
        for i in range(3)
    ]
    NPS = 4  # PSUM rotation depth: 4 duo-buffers x 2 banks = all 8 banks
    DUO = 2 * CHUNK  # 1024 fp32 = 2 banks
    ps = [
        ctx.enter_context(nc.psum_tensor(f"ps{i}", [128, DUO], f32))
        for i in range(NPS)
    ]

    s_load = ctx.enter_context(nc.semaphore("s_load"))
    s_mm = ctx.enter_context(nc.semaphore("s_mm"))
    s_act = ctx.enter_context(nc.semaphore("s_act"))
    s_chain = ctx.enter_context(nc.semaphore("s_chain"))
    s_f1 = ctx.enter_context(nc.semaphore("s_f1"))
    s_g = ctx.enter_context(nc.semaphore("s_g"))
    s_dma = ctx.enter_context(nc.semaphore("s_dma"))

    with nc.Block() as block:

        # duo ownership: ACT copies duos 0,1,2,3,5; DVE chain-folds duos
        # 4,6,7 from PSUM straight into f1 slots.  The f1 pairing is
        # (chunk c, chunk c+8): (0,8),(1,9) <- chain duo4; (2,10),(3,11)
        # <- fp16 merge of two ACT-written duos; (4,12),(5,13) <- duo6;
        # (6,14),(7,15) <- duo7.
        def pe_wait(eng, Dp):
            tp, dp = divmod(Dp, 8)
            if dp <= 3:
                eng.wait_ge(s_act, 5 * tp + dp + 1)
            elif dp == 5:
                eng.wait_ge(s_act, 5 * tp + 5)
            elif dp == 4:
                eng.wait_ge(s_chain, 3 * tp + 1)
            elif dp == 6:
                eng.wait_ge(s_chain, 3 * tp + 2)
            else:
                eng.wait_ge(s_chain, 3 * tp + 3)

        @block.tensor
        def _(eng):
            for t in range(T):
                ztile = zt_all[:, t * 128 : (t + 1) * 128]
                for d in range(8):
                    D = 8 * t + d
                    if t == 0 and d % 2 == 0:
                        # zT + wT pieces loaded incrementally (16 per DMA)
                        eng.wait_ge(s_load, 16 * (d // 2 + 2))
                    if D >= NPS:
                        pe_wait(eng, D - NPS)
                    for c2 in range(2):
                        c = d * 2 + c2
                        mm = nc.tensor.matmul(
                            ps[D % NPS][:, c2 * CHUNK : (c2 + 1) * CHUNK],
                            ztile,
                            wt_all[:, c * CHUNK : (c + 1) * CHUNK],
                            start=True,
                            stop=True,
                        )
                    mm.then_inc(s_mm, 1)

        @block.scalar
        def _(eng):
            for t in range(T):
                x = xa[t % 3]
                if t >= 3:
                    eng.wait_ge(s_f1, t - 2)
                for d in (0, 1, 2, 3, 5):
                    D = 8 * t + d
                    eng.wait_ge(s_mm, D + 1)
                    nc.scalar.copy(
                        x[:, d * DUO : (d + 1) * DUO], ps[D % NPS][:]
                    ).then_inc(s_act, 1)

        @block.vector
        def _(eng):
            for t in range(T):
                x = xa[t % 3]
                # chain duo4: psum chunks 8,9 onto xa chunks 0,1 -> f1[0:1024]
                eng.wait_ge(s_mm, 8 * t + 5)
                eng.wait_ge(s_act, 5 * t + 1)
                nc.vector.tensor_max(
                    f1[:, :DUO], ps[(8 * t + 4) % NPS][:], x[:, :DUO]
                ).then_inc(s_chain, 1)
                # chain duo6: chunks 12,13 onto xa chunks 4,5 -> f1[2048:3072]
                eng.wait_ge(s_mm, 8 * t + 7)
                eng.wait_ge(s_act, 5 * t + 3)
                nc.vector.tensor_max(
                    f1[:, 2 * DUO : 3 * DUO], ps[(8 * t + 6) % NPS][:], x[:, 2 * DUO : 3 * DUO]
                ).then_inc(s_chain, 1)
                # chain duo7: chunks 14,15 onto xa chunks 6,7 -> f1[3072:4096]
                eng.wait_ge(s_mm, 8 * t + 8)
                eng.wait_ge(s_act, 5 * t + 4)
                nc.vector.tensor_max(
                    f1[:, 3 * DUO : 4 * DUO], ps[(8 * t + 7) % NPS][:], x[:, 3 * DUO : 4 * DUO]
                ).then_inc(s_chain, 1)
                # fp16 merge: (chunks 2,3) vs (chunks 10,11) -> f1[1024:2048]
                eng.wait_ge(s_act, 5 * t + 5)
                nc.vector.tensor_max(
                    f1[:, DUO : 2 * DUO], x[:, DUO : 2 * DUO], x[:, 5 * DUO : 6 * DUO]
                ).then_inc(s_f1, 1)
                if t >= 3:
                    eng.wait_ge(s_dma, 16 * (t - 2))
                nc.vector.tensor_max(
                    gt[t % 3][:], f1[:, :NGROUPS], f1[:, NGROUPS:]
                ).then_inc(s_g, 1)

        @block.sync
        def _(eng):
            eng.dma_start(zt_all[:], zT_in[:]).then_inc(s_load, 16)
            for piece in range(4):
                eng.dma_start(
                    wt_all[:, piece * 2048 : (piece + 1) * 2048],
                    wT_in[:, piece * 2048 : (piece + 1) * 2048],
                ).then_inc(s_load, 16)
            for t in range(T):
                eng.wait_ge(s_g, t + 1)
                eng.dma_start(
                    g_out[t * 128 : (t + 1) * 128, :], gt[t % 3][:]
                ).then_inc(s_dma, 16)

    nc.finalize()
    ctx.close()
    return nc


def _get_nc():
    if "nc" not in _cached:
        _cached["nc"] = _build_nc()
    return _cached["nc"]


def _host_prep(z_real, z_imag, weight):
    z = np.concatenate(
        [np.asarray(z_real, np.float32), np.asarray(z_imag, np.float32)], axis=1
    )  # [N, 128]
    w = np.asarray(weight, np.float32)
    y64 = (w.astype(np.float64) ** 2).sum(1)  # [K]
    order = np.argsort(y64, kind="stable")
    ws = w[order]  # [K, 128] sorted by ||w||^2
    ys = y64[order]

    u = np.arange(K)
    slots = (u % GROUP) * NGROUPS + (u // GROUP)  # code u -> score slot
    wT_dev = np.empty((D2, K), np.float16)
    wT_dev[:, slots] = ws.T.astype(np.float16)
    zT = np.ascontiguousarray(z.T).astype(np.float16)  # [128, N]
    return z, w, order, ws, ys, wT_dev, zT


def _device_groupmax(nc, zT, wT_dev):
    from concourse.bass_utils import run_bass_kernel_spmd

    in_maps = [
        {
            "zT": np.ascontiguousarray(
                zT[:, c * TOK_PER_CORE : (c + 1) * TOK_PER_CORE]
            ),
            "wT": wT_dev,
        }
        for c in range(N_CORES)
    ]
    res = run_bass_kernel_spmd(nc, in_maps, list(range(N_CORES)))
    G = np.concatenate([r["g_out"] for r in res.results], axis=0)
    return G.astype(np.float32)  # [N, 512]


def _host_select(z, order, ws, ys, G):
    """Filter candidate groups per token and rescore exactly in fp64."""
    n = z.shape[0]
    y_grp = ys.reshape(NGROUPS, GROUP)
    y_lo = y_grp[:, 0]  # sorted ascending within group
    y_hi = y_grp[:, -1]

    U = G + MARGIN - (y_lo * 0.5)[None, :]
    L = G - MARGIN - (y_hi * 0.5)[None, :]
    bestL = L.max(axis=1)
    cand = U >= bestL[:, None]

    tok_o, grp_o = np.nonzero(cand)  # ~1.1 pairs per token

    zf = z.astype(np.float64)
    wsf = ws.astype(np.float64)
    w_blk = wsf.reshape(NGROUPS, GROUP, D2)[grp_o]  # [P, GROUP, 128]
    sc = np.matmul(w_blk, zf[tok_o][:, :, None])[:, :, 0]  # [P, GROUP]
    sc -= 0.5 * ys.reshape(NGROUPS, GROUP)[grp_o]
    mx = sc.max(axis=1)
    orig = order.reshape(NGROUPS, GROUP)[grp_o]  # original code ids
    tied = sc == mx[:, None]
    jj = np.where(tied, orig, np.iinfo(np.int64).max).min(axis=1)

    best_val = np.full(n, -np.inf)
    np.maximum.at(best_val, tok_o, mx)
    is_best = mx == best_val[tok_o]
    best_j = np.full(n, np.iinfo(np.int64).max)
    np.minimum.at(best_j, tok_o[is_best], jj[is_best])
    return best_j.astype(np.int64)


def kernel(z_real, z_imag, weight):
    z, w, order, ws, ys, wT_dev, zT = _host_prep(z_real, z_imag, weight)
    nc = _get_nc()
    G = _device_groupmax(nc, zT, wT_dev)
    idx = _host_select(z, order, ws, ys, G)

    z_q = w[idx]  # [N, 128] fp32
    z_q_c = (z_q[:, :DIM] + 1j * z_q[:, DIM:]).astype(np.complex64)
    diff = z_q.astype(np.float64) - z.astype(np.float64)
    loss_sample = (1.25 * (diff**2).mean(axis=1)).astype(np.float32)
    indices = idx.astype(np.int32)
    counts = np.bincount(idx, minlength=K).astype(np.float64)
    avg_probs = counts / z.shape[0]
    batch_entropy = np.float32(-(avg_probs * np.log(avg_probs + 1e-10)).sum())
    return z_q_c, loss_sample, indices, batch_entropy
